# revision 1
# baseline (speedup 1.0000x reference)
"""Trainium2 Bass kernel for nn_LoopVisibleLSTM (T=2048, B=32, D=256, H=256, L=2).

Architecture: the time recurrence is inherently sequential, so one core runs
the whole recurrence with the full batch (B=32).  Per 32-step half-block the
input projection x = input @ W_init.T + b_init is bulk-computed (PE
transposes + matmuls); per 4-step group the input-side gate projection
G0in = x @ Wih0.T + bias0 is bulk-matmul'ed directly into PSUM, packing the
4 steps x 32 batch rows into the 128 PSUM partitions.  Each sequential step
then issues one fp32r matmul (h.T stationary [128,32], Whh.T moving, N=512
chunks) that accumulates onto the bulk PSUM, activations on ACT, elementwise
on DVE, and PE transposes to produce the next step's h.T.
The "backward" half of the module's output is (faithfully to the reference)
just the final forward hidden state broadcast over time, assembled on host.
"""

import sys
import os

for _p in ("/opt/pypackages", "/opt/trn_rl_repo"):
    if _p not in sys.path:
        sys.path.insert(0, _p)

import numpy as np

T_FULL, B, D, H = 2048, 32, 256, 256
G4 = 4            # steps packed per PSUM group
HALF = 32         # steps per half-block (bulk x granularity)
BODY = 64         # steps per For_i body (2 half-blocks)
FP32 = None       # filled after imports
F32R = None


def build(T):
    """Build the Bass program for a T-step run. Returns nc."""
    import concourse.bass as bass
    import concourse.mybir as mybir
    import concourse.tile as tile
    from concourse import bacc
    from concourse.bass import AP  # noqa: F401

    global FP32, F32R
    FP32 = mybir.dt.float32
    F32R = mybir.dt.float32r
    AF = mybir.ActivationFunctionType

    assert T % BODY == 0
    n_body = T // BODY

    nc = bacc.Bacc("TRN2", target_bir_lowering=False, debug=False)

    # ---------------- DRAM parameters ----------------
    inp = nc.declare_dram_parameter("input", [T * B, D], FP32, isOutput=False)
    whh0t_d = nc.declare_dram_parameter("whh0t", [256, 1024], F32R, isOutput=False)
    wih0t_d = nc.declare_dram_parameter("wih0t", [256, 1024], F32R, isOutput=False)
    whh1t_d = nc.declare_dram_parameter("whh1t", [256, 1024], F32R, isOutput=False)
    wih1t_d = nc.declare_dram_parameter("wih1t", [256, 1024], F32R, isOutput=False)
    winitt_d = nc.declare_dram_parameter("winitt", [256, 256], F32R, isOutput=False)
    bias0_d = nc.declare_dram_parameter("bias0", [1, 1024], F32R, isOutput=False)
    bias1_d = nc.declare_dram_parameter("bias1", [1, 1024], F32R, isOutput=False)
    binit_d = nc.declare_dram_parameter("binit", [1, 256], F32R, isOutput=False)
    ones_d = nc.declare_dram_parameter("ones", [1, 512], F32R, isOutput=False)
    zeros_d = nc.declare_dram_parameter("zeros128", [128, 128], F32R, isOutput=False)
    id128_d = nc.declare_dram_parameter("id128", [128, 128], FP32, isOutput=False)
    h0t_init_d = nc.declare_dram_parameter("h0t_init", [256, 32], F32R, isOutput=False)
    h1t_init_d = nc.declare_dram_parameter("h1t_init", [256, 32], F32R, isOutput=False)
    c0_init_d = nc.declare_dram_parameter("c0_init", [32, 256], FP32, isOutput=False)
    c1_init_d = nc.declare_dram_parameter("c1_init", [32, 256], FP32, isOutput=False)
    fwd = nc.declare_dram_parameter("fwd", [T * B, H], FP32, isOutput=True)

    ctxs = []

    def sb(shape, dtype=None):
        cm = nc.sbuf_tensor(shape, dtype or FP32)
        t = cm.__enter__()
        ctxs.append(cm)
        return t

    def ps(shape, dtype=None):
        cm = nc.psum_tensor(shape, dtype or FP32)
        t = cm.__enter__()
        ctxs.append(cm)
        return t

    # ---------------- SBUF constants ----------------
    whh0t = [sb([128, 1024], F32R) for _ in range(2)]
    wih0t = [sb([128, 1024], F32R) for _ in range(2)]
    whh1t = [sb([128, 1024], F32R) for _ in range(2)]
    wih1t = [sb([128, 1024], F32R) for _ in range(2)]
    winitt = [sb([128, 256], F32R) for _ in range(2)]
    bias0 = sb([1, 1024], F32R)
    bias1 = sb([1, 1024], F32R)
    binit = sb([1, 256], F32R)
    ones = sb([1, 512], F32R)
    id128 = sb([128, 128])

    # ---------------- SBUF working buffers ----------------
    # input block (untransposed), per half-block ping-pong: 8 squares of
    # [128 rows, 256 dims] side by side
    inblk = [sb([128, 8 * 256]) for _ in range(2)]
    # input.T block: [256 dims -> 2 tiles][hb] of [128, 1024 (t,b) cols]
    inT = [[sb([128, 1024], F32R) for _ in range(2)] for _ in range(2)]  # [k][hb]
    # x.T block: same geometry
    xT = [[sb([128, 1024], F32R) for _ in range(2)] for _ in range(2)]  # [k][hb]
    # h.T group buffers [gb][k]: [128, 128] = 4 steps x 32 batch cols
    # (layer-0 only; feeds the bulk G1in matmul)
    h0t = [[sb([128, 128], F32R) for _ in range(2)] for _ in range(2)]  # [gb][k]
    # zero-padded stationary tiles for the step matmuls: Z[layer][pos][k] is
    # all-zero except column block `pos`, which holds h.T of step t with
    # (t+1) % 4 == pos.  The step matmul then targets all 128 PSUM
    # partitions at offset 0 (fp32r + tile_position column offsets is
    # rejected by the ISA), adding zero to the other steps' rows.
    Z = [[[sb([128, 128], F32R) for _ in range(2)] for _ in range(G4)]
         for _ in range(2)]
    # cell state ping-pong
    c0_ = [sb([32, 256]) for _ in range(2)]
    c1_ = [sb([32, 256]) for _ in range(2)]
    # elementwise scratch, per layer x parity
    sif = [[sb([32, 1024]) for _ in range(2)] for _ in range(2)]  # [l][p]
    fc = [[sb([32, 256]) for _ in range(2)] for _ in range(2)]
    ig = [[sb([32, 256]) for _ in range(2)] for _ in range(2)]
    tcc = [[sb([32, 256]) for _ in range(2)] for _ in range(2)]
    h0v = [sb([32, 256]) for _ in range(2)]  # layer0 hidden (untransposed)
    # output block per half-block: [32 batch parts, 32 steps * 256]
    outblk = [sb([32, HALF * 256]) for _ in range(2)]

    # ---------------- PSUM ----------------
    g0p = [ps([128, 1024]) for _ in range(2)]   # 4 banks
    g1p = ps([128, 1024])                        # 2 banks
    scrA = ps([128, 512])                        # 1 bank: h-transposes + input-T
    scrB = ps([128, 512])                        # 1 bank: x-MM chunks

    import concourse.tile as tile_mod

    with tile_mod.TileContext(nc) as tc:
        dma = nc.sync

        # ------------ constant + init loads ------------
        for k in range(2):
            dma.dma_start(whh0t[k][:, :], whh0t_d[128 * k:128 * (k + 1), :])
            dma.dma_start(wih0t[k][:, :], wih0t_d[128 * k:128 * (k + 1), :])
            dma.dma_start(whh1t[k][:, :], whh1t_d[128 * k:128 * (k + 1), :])
            dma.dma_start(wih1t[k][:, :], wih1t_d[128 * k:128 * (k + 1), :])
            dma.dma_start(winitt[k][:, :], winitt_d[128 * k:128 * (k + 1), :])
        dma.dma_start(bias0[:, :], bias0_d[:, :])
        dma.dma_start(bias1[:, :], bias1_d[:, :])
        dma.dma_start(binit[:, :], binit_d[:, :])
        dma.dma_start(ones[:, :], ones_d[:, :])
        dma.dma_start(id128[:, :], id128_d[:, :])
        # zero the padded stationary tiles once; non-pos blocks stay zero
        # (DMA from a DRAM zeros constant: DVE memset cannot emit f32r)
        for l in range(2):
            for pos in range(G4):
                for k in range(2):
                    dma.dma_start(Z[l][pos][k][:, :], zeros_d[:, :])
        # initial h.T: consumed by step t=0 from Z[l][0], column block 0
        for k in range(2):
            dma.dma_start(Z[0][0][k][:, 0:32], h0t_init_d[128 * k:128 * (k + 1), :])
            dma.dma_start(Z[1][0][k][:, 0:32], h1t_init_d[128 * k:128 * (k + 1), :])
        dma.dma_start(c0_[0][:, :], c0_init_d[:, :])
        dma.dma_start(c1_[0][:, :], c1_init_d[:, :])

        def emit_bulk_x(i, hb):
            """input DMA, transpose to input.T, x.T = W_init@input.T + b_init."""
            # one 3D DMA for the whole [1024 rows, 256] block
            src = inp[bass.ds(i + hb * (HALF * B), 1024), :].rearrange(
                "(r p) d -> p r d", p=128)
            dma.dma_start(inblk[hb][:, :].rearrange("p (r d) -> p r d", r=8), src)
            # 16 square transposes [128,128] -> inT
            for rr in range(8):
                for cdim in range(2):
                    sl = scrA[:, 128 + 128 * ((rr * 2 + cdim) % 2):
                              256 + 128 * ((rr * 2 + cdim) % 2)]
                    nc.tensor.transpose(
                        sl,
                        inblk[hb][:, 256 * rr + 128 * cdim:256 * rr + 128 * (cdim + 1)],
                        id128[:, :],
                    )
                    # evac: alternate ACT / DVE
                    dst = inT[cdim][hb][:, 128 * rr:128 * (rr + 1)]
                    if (rr + cdim) % 2 == 0:
                        nc.scalar.copy(dst, sl)
                    else:
                        nc.vector.tensor_copy(dst, sl)
            # x.T = W_init @ input.T + b_init, in [128,256] chunks
            for m in range(2):
                for cc in range(4):
                    out = scrB[:, 256 * (cc % 2):256 * (cc % 2 + 1)]
                    # bias ride: out = b_init[m-slice].T outer ones
                    nc.tensor.matmul(
                        out, binit[:, 128 * m:128 * (m + 1)],
                        ones[:, 0:256], start=True, stop=False,
                    )
                    for k in range(2):
                        nc.tensor.matmul(
                            out,
                            winitt[k][:, 128 * m:128 * (m + 1)],
                            inT[k][hb][:, 256 * cc:256 * (cc + 1)],
                            start=False, stop=(k == 1),
                        )
                    nc.scalar.copy(xT[m][hb][:, 256 * cc:256 * (cc + 1)], out)

        def emit_g0in(hb, g_loc, pp):
            """Bulk G0in for group: bias0 + x @ Wih0.T into g0p[pp]."""
            for c in range(2):
                out = g0p[pp][:, 512 * c:512 * (c + 1)]
                nc.tensor.matmul(out, ones[:, 0:128],
                                 bias0[:, 512 * c:512 * (c + 1)],
                                 start=True, stop=False)
                for k in range(2):
                    nc.tensor.matmul(
                        out,
                        xT[k][hb][:, 128 * g_loc:128 * (g_loc + 1)],
                        wih0t[k][:, 512 * c:512 * (c + 1)],
                        start=False, stop=(k == 1),
                    )

        def emit_g1in(pp):
            """Bulk G1in for group: bias1 + h0(group) @ Wih1.T into g1p."""
            for c in range(2):
                out = g1p[:, 512 * c:512 * (c + 1)]
                nc.tensor.matmul(out, ones[:, 0:128],
                                 bias1[:, 512 * c:512 * (c + 1)],
                                 start=True, stop=False)
                for k in range(2):
                    nc.tensor.matmul(
                        out,
                        h0t[pp][k][:, 0:128],
                        wih1t[k][:, 512 * c:512 * (c + 1)],
                        start=False, stop=(k == 1),
                    )

        def base_off(layer):
            return 0 if layer == 0 else 64

        def emit_step(layer, t_loc, hb, g_loc, j, pp):
            """One recurrent step for one layer."""
            p = t_loc % 2
            whht = whh0t if layer == 0 else whh1t
            gp = g0p[pp] if layer == 0 else g1p
            cc_ = c0_ if layer == 0 else c1_
            rows = slice(32 * j, 32 * (j + 1))

            # step matmul: h_{t-1}.T sits in column block j of the
            # zero-padded stationary Z[layer][j]; all other columns are 0,
            # so accumulating over all 128 partitions only updates rows j.
            for c in range(2):
                for k in range(2):
                    nc.tensor.matmul(
                        gp[:, 512 * c:512 * (c + 1)],
                        Z[layer][j][k][:, :],
                        whht[k][:, 512 * c:512 * (c + 1)],
                        start=False, stop=(k == 1), skip_group_check=True,
                    )
            # one sigmoid over all four gates; the g-gate's weights/bias
            # are pre-scaled by 2 on host so tanh(g) = 2*sig(2g) - 1 folds
            # into the DVE ops below.  Gate order [i f g o].
            AFt = AF
            s_ = sif[layer][p]
            nc.scalar.activation(s_[:, :], gp[rows, :], AFt.Sigmoid)
            # cell update: c = f*c + i*(2*sig(2g) - 1)
            c_prev = cc_[t_loc % 2]
            c_new = cc_[(t_loc + 1) % 2]
            nc.vector.tensor_mul(fc[layer][p][:, :], s_[:, 256:512], c_prev[:, :])
            nc.vector.scalar_tensor_tensor(
                ig[layer][p][:, :], s_[:, 512:768], 0.5, s_[:, 0:256],
                mybir.AluOpType.subtract, mybir.AluOpType.mult)
            nc.vector.scalar_tensor_tensor(
                c_new[:, :], ig[layer][p][:, :], 2.0, fc[layer][p][:, :],
                mybir.AluOpType.mult, mybir.AluOpType.add)
            nc.scalar.activation(tcc[layer][p][:, :], c_new[:, :], AFt.Tanh)
            # hidden
            if layer == 0:
                hv = h0v[p]
            else:
                hv = outblk[hb][:, 256 * (g_loc * G4 + j):256 * (g_loc * G4 + j + 1)]
            nc.vector.tensor_mul(hv[:, :], s_[:, 768:1024], tcc[layer][p][:, :])
            # transpose h -> h.T slices (2 halves of 128); the consumer of
            # h.T(t) is step t+1, which reads Z[layer][(t+1) % 4] block
            # (t+1) % 4.  Layer-0 h.T additionally feeds the bulk G1in
            # matmul via the contiguous group buffer h0t.
            nxt = (j + 1) % G4
            for k in range(2):
                sl = scrA[:, base_off(layer) + 32 * k:base_off(layer) + 32 * (k + 1)]
                nc.tensor.transpose(sl, hv[:, 128 * k:128 * (k + 1)], id128[0:32, 0:32])
                nc.vector.tensor_copy(Z[layer][nxt][k][:, 32 * nxt:32 * (nxt + 1)], sl)
                if layer == 0:
                    # group buffer for the bulk G1in matmul; gpsimd reads the
                    # SBUF Z block (gpsimd cannot read PSUM), keeping the
                    # copy off the ACT/DVE chains
                    nc.gpsimd.tensor_copy(
                        h0t[pp][k][:, 32 * j:32 * (j + 1)],
                        Z[0][nxt][k][:, 32 * nxt:32 * (nxt + 1)])

        def emit_out_dma(i, hb):
            src = outblk[hb][:, :].rearrange("b (t d) -> b t d", t=HALF)
            dst = fwd[bass.ds(i + hb * (HALF * B), HALF * B), :].rearrange(
                "(t b) d -> b t d", b=32)
            dma.dma_start(dst, src)

        def emit_body(i):
            # software pipeline: layer 1 lags layer 0 by one 4-step group so
            # the two dependence chains interleave on the engines.
            n_groups = BODY // G4  # 16
            for g in range(n_groups + 1):
                gl = g - 1          # lagged group for layer 1
                if g < n_groups:
                    hb = g // 8
                    g_loc = g % 8
                    if g_loc == 0:
                        emit_bulk_x(i, hb)
                    emit_g0in(hb, g_loc, g % 2)
                if gl >= 0:
                    # G1in(gl): h0t(gl) is complete; the WAR on g1p against
                    # L1(gl-1)'s reads resolved a full group ago
                    emit_g1in(gl % 2)
                for j in range(G4):
                    if g < n_groups:
                        emit_step(0, g * G4 + j, g // 8, g % 8, j, g % 2)
                    if gl >= 0:
                        emit_step(1, gl * G4 + j, gl // 8, gl % 8, j, gl % 2)
                if gl >= 0 and gl % 8 == 7:
                    emit_out_dma(i, gl // 8)

        if n_body == 1:
            emit_body(0)
        else:
            with tc.For_i(0, T * B, BODY * B) as i:
                emit_body(i)

    for cm in reversed(ctxs):
        cm.__exit__(None, None, None)

    nc.compile()
    return nc


def rne11(x):
    """Round fp32 to f32r: round-to-nearest-even keeping 11 mantissa bits."""
    xi = np.ascontiguousarray(x, np.float32).view(np.uint32).astype(np.uint64)
    shift = 12
    half = np.uint64(1 << (shift - 1))
    lsb = (xi >> np.uint64(shift)) & np.uint64(1)
    r = ((xi + half - np.uint64(1) + lsb) >> np.uint64(shift)) << np.uint64(shift)
    return (r & np.uint64(0xFFFFFFFF)).astype(np.uint32).view(np.float32).reshape(np.shape(x))


def prep_inputs(inputs, T):
    """Host-side input re-layout (cheap: weights only; input passed as-is)."""
    inp = np.ascontiguousarray(inputs["input"], dtype=np.float32)
    Wih = inputs["Wih"].astype(np.float32)
    Whh = inputs["Whh"].astype(np.float32)
    bih = inputs["bih"].astype(np.float32)
    bhh = inputs["bhh"].astype(np.float32)
    W_init = inputs["W_init"].astype(np.float32)
    b_init = inputs["b_init"].astype(np.float32)
    h0 = inputs["h0"].astype(np.float32)
    c0 = inputs["c0"].astype(np.float32)

    def g2(wt):
        w = np.ascontiguousarray(wt, np.float32).copy()
        w[:, 512:768] *= 2.0
        return w

    im = {
        "input": inp.reshape(T * B, D),
        "whh0t": rne11(g2(Whh[0].T)),
        "wih0t": rne11(g2(Wih[0].T)),
        "whh1t": rne11(g2(Whh[1].T)),
        "wih1t": rne11(g2(Wih[1].T)),
        "winitt": rne11(W_init.T),
        "bias0": rne11(g2((bih[0] + bhh[0]).reshape(1, 1024))),
        "bias1": rne11(g2((bih[1] + bhh[1]).reshape(1, 1024))),
        "binit": rne11(b_init.reshape(1, 256)),
        "ones": np.ones((1, 512), np.float32),
        "zeros128": np.zeros((128, 128), np.float32),
        "id128": np.eye(128, dtype=np.float32),
        "h0t_init": rne11(np.ascontiguousarray(h0[0].T)),
        "h1t_init": rne11(np.ascontiguousarray(h0[1].T)),
        "c0_init": np.ascontiguousarray(c0[0]),
        "c1_init": np.ascontiguousarray(c0[1]),
    }
    return im


def run_device(inputs, T, trace=False, repeats=0):
    """Run on hardware. trace/repeats: rerun the compiled NEFF to get a
    warm-execution wall time (NTFF profiling is unavailable under axon)."""
    import time
    from concourse import bass_utils

    nc = build(T)
    im = prep_inputs(inputs, T)
    res = bass_utils.run_bass_kernel_spmd(nc, [im], [0])
    times = []
    if trace or repeats:
        for _ in range(max(repeats, 3)):
            t0 = time.time()
            res = bass_utils.run_bass_kernel_spmd(nc, [im], [0])
            times.append(time.time() - t0)
        res.exec_time_ns = int(min(times) * 1e9)
    fwd = res.results[0]["fwd"].reshape(T, B, H)
    return fwd, res


def kernel(**inputs):
    T = inputs["input"].shape[0]
    fwd, _ = run_device(inputs, T)
    out = np.empty((T, B, 2 * H), dtype=np.float32)
    out[:, :, :H] = fwd
    out[:, :, H:] = fwd[-1][None]
    return out


if __name__ == "__main__":
    # quick CoreSim smoke test with small T
    import concourse.bass as bass  # noqa
    from concourse.bass_interp import CoreSim

    T = int(os.environ.get("SIM_T", "64"))
    rng = np.random.default_rng(0)
    k = 1.0 / np.sqrt(H)
    inputs = {
        "input": rng.standard_normal((T, B, D), dtype=np.float32),
        "W_init": rng.uniform(-k, k, (H, D)).astype(np.float32),
        "b_init": rng.uniform(-k, k, (H,)).astype(np.float32),
        "Wih": rng.uniform(-k, k, (2, 4 * H, H)).astype(np.float32),
        "Whh": rng.uniform(-k, k, (2, 4 * H, H)).astype(np.float32),
        "bih": rng.uniform(-k, k, (2, 4 * H)).astype(np.float32),
        "bhh": rng.uniform(-k, k, (2, 4 * H)).astype(np.float32),
        "h0": rng.uniform(-k, k, (2, B, H)).astype(np.float32),
        "c0": rng.uniform(-k, k, (2, B, H)).astype(np.float32),
    }

    # numpy reference
    def np_ref(inp):
        x_all = inp["input"]
        h = inp["h0"].copy()
        c = inp["c0"].copy()
        outs = []
        for t in range(T):
            x = x_all[t] @ inp["W_init"].T + inp["b_init"]
            for l in range(2):
                gates = x @ inp["Wih"][l].T + inp["bih"][l] + h[l] @ inp["Whh"][l].T + inp["bhh"][l]
                i_, f_, g_, o_ = np.split(gates, 4, axis=-1)
                i_ = 1 / (1 + np.exp(-i_)); f_ = 1 / (1 + np.exp(-f_))
                o_ = 1 / (1 + np.exp(-o_)); g_ = np.tanh(g_)
                c[l] = f_ * c[l] + i_ * g_
                h[l] = o_ * np.tanh(c[l])
                x = h[l]
            outs.append(h[1].copy())
        return np.stack(outs)

    expected = np_ref(inputs)

    nc = build(T)

    sim = CoreSim(nc)
    im = prep_inputs(inputs, T)
    for name, arr in im.items():
        sim.tensor(name)[:] = arr
    sim.simulate()
    got = sim.tensor("fwd").reshape(T, B, H)
    err = np.abs(got - expected).max() / (np.abs(expected).max() + 1e-9)
    print("SIM max-rel err:", err)
    print("sample got", got[0, 0, :4], "exp", expected[0, 0, :4])



# revision 2
# speedup vs baseline: 772.8407x; 772.8407x over previous
"""Trainium2 Bass kernel for nn_LoopVisibleLSTM (T=2048, B=32, D=256, H=256, L=2).

Transposed-state design: all recurrent state is kept hidden-dim-major
([128, cols]) so every elementwise op uses all 128 lanes and no per-step
transposes sit on the critical path.

 - gates.T per step live in PSUM as [128, 8 chunks x 32 batch]; the
   recurrent h @ Whh.T contribution is 16 tiny matmuls (stationary =
   Whh.T chunk [128,128], moving = h.T k-slice [128,32]).
 - input-side gate projections are bulk-computed 8 steps at a time into
   PSUM ([128, 256] per gate-chunk), evacuated to SBUF with the bias
   pre-seeded via a [1,128] x [1,256] ones-matmul, then seeded into the
   step's gate PSUM with one identity matmul (256 cols).
 - layer 0's input path is fused on host: M0 = Wih0 @ W_init,
   beta0 = Wih0 @ b_init + bih0 + bhh0, so the x projection disappears.
 - layer 1 lags layer 0 by 8 steps; its input bulk consumes h0.T straight
   from a 16-slot ring buffer that the layer-0 h-mul writes.
 - tanh(g) is folded into the single sigmoid via the 2*sig(2x)-1 trick
   (g-gate weights/bias pre-scaled by 2 on host), so each layer-step does
   one sigmoid [128,256] + one tanh [128,64] on ACT.
The "backward" half of the module's output is (faithfully to the
reference) the final forward hidden state broadcast over time, assembled
on host.
"""

import sys
import os

for _p in ("/opt/pypackages", "/opt/trn_rl_repo"):
    if _p not in sys.path:
        sys.path.insert(0, _p)

import numpy as np

T_FULL, B, D, H = 2048, 32, 256, 256
SB = 8            # steps per sub-block (input bulk granularity, L1 lag)
HALF = 32         # steps per half-block (DMA granularity)
BODY = 64         # steps per For_i body
NSLOT = 16        # h.T ring slots


def build(T, reps=1):
    """Build the Bass program for a T-step run. Returns nc.

    reps > 1 re-emits the whole recurrence (same DRAM I/O) for
    differential device-time measurement: exec(T) ~ (wall(R) - wall(1))
    / (R - 1).
    """
    import concourse.bass as bass
    import concourse.mybir as mybir
    from concourse import bacc

    FP32 = mybir.dt.float32
    F32R = mybir.dt.float32r
    BF16 = mybir.dt.bfloat16
    AF = mybir.ActivationFunctionType

    assert T % BODY == 0
    n_body = T // BODY

    nc = bacc.Bacc("TRN2", target_bir_lowering=False, debug=False)

    # ---------------- DRAM parameters ----------------
    inp = nc.declare_dram_parameter("input", [T * B, D], FP32, isOutput=False)
    m0t_d = nc.declare_dram_parameter("m0t", [256, 1024], F32R, isOutput=False)
    whh0t_d = nc.declare_dram_parameter("whh0t", [256, 1024], BF16, isOutput=False)
    whh1t_d = nc.declare_dram_parameter("whh1t", [256, 1024], BF16, isOutput=False)
    wih1t_d = nc.declare_dram_parameter("wih1t", [256, 1024], BF16, isOutput=False)
    beta0_d = nc.declare_dram_parameter("beta0", [1, 1024], F32R, isOutput=False)
    beta1_d = nc.declare_dram_parameter("beta1", [1, 1024], F32R, isOutput=False)
    ones_d = nc.declare_dram_parameter("ones", [1, 512], F32R, isOutput=False)
    id128f_d = nc.declare_dram_parameter("id128f", [128, 128], FP32, isOutput=False)
    id128r_d = nc.declare_dram_parameter("id128r", [128, 128], F32R, isOutput=False)
    id128b_d = nc.declare_dram_parameter("id128b", [128, 128], BF16, isOutput=False)
    h0t_init_d = nc.declare_dram_parameter("h0t_init", [256, 32], BF16, isOutput=False)
    h1t_init_d = nc.declare_dram_parameter("h1t_init", [256, 32], BF16, isOutput=False)
    c0t_init_d = nc.declare_dram_parameter("c0t_init", [128, 64], FP32, isOutput=False)
    c1t_init_d = nc.declare_dram_parameter("c1t_init", [128, 64], FP32, isOutput=False)
    fwd = nc.declare_dram_parameter("fwd", [T * B, H], FP32, isOutput=True)

    ctxs = []

    def sb_(shape, dtype=None):
        cm = nc.sbuf_tensor(shape, dtype or FP32)
        t = cm.__enter__()
        ctxs.append(cm)
        return t

    def ps_(shape, dtype=None):
        cm = nc.psum_tensor(shape, dtype or FP32)
        t = cm.__enter__()
        ctxs.append(cm)
        return t

    # ---------------- SBUF constants ----------------
    m0t = [sb_([128, 1024], F32R) for _ in range(2)]
    whh0t = [sb_([128, 1024], BF16) for _ in range(2)]
    whh1t = [sb_([128, 1024], BF16) for _ in range(2)]
    wih1t = [sb_([128, 1024], BF16) for _ in range(2)]
    beta0 = sb_([1, 1024], F32R)
    beta1 = sb_([1, 1024], F32R)
    ones = sb_([1, 512], F32R)
    id128f = sb_([128, 128], FP32)
    id128r = sb_([128, 128], F32R)
    id128b = sb_([128, 128], BF16)

    # ---------------- SBUF working buffers ----------------
    # input block per half-block (pp): 8 squares of [128 rows, 256 dims]
    inblk = [sb_([128, 8 * 256]) for _ in range(2)]
    # input.T per half-block (pp): [k][hb] of [128, 1024 (t,b) cols]
    intp = [[sb_([128, 1024], F32R) for _ in range(2)] for _ in range(2)]
    # bulk input-side gates, transposed, per sub-block (pp):
    # col layout (t_loc, m, b) = t_loc*256 + m*32 + b
    gin0 = [sb_([128, 8 * 256], F32R) for _ in range(2)]
    gin1 = [sb_([128, 8 * 256], F32R) for _ in range(2)]
    # h.T rings: col (k, s, b) = k*512 + s*32 + b, slot s = t % 16
    h0ring = sb_([128, 1024], BF16)
    h1ring = sb_([128, 1024], BF16)
    # c.T state (pp): col (k, b) = k*32 + b
    cT0 = [sb_([128, 64]) for _ in range(2)]
    cT1 = [sb_([128, 64]) for _ in range(2)]
    # sigmoid outputs (pp): [128, 8 chunks x 32]
    s0 = [sb_([128, 256]) for _ in range(2)]
    s1 = [sb_([128, 256]) for _ in range(2)]
    # elementwise scratch per layer x parity
    fc = [[sb_([128, 64]) for _ in range(2)] for _ in range(2)]
    ig = [[sb_([128, 64]) for _ in range(2)] for _ in range(2)]
    tc = [[sb_([128, 64]) for _ in range(2)] for _ in range(2)]
    # output block per half-block (pp): [32 batch parts, 32 steps * 256]
    outblk = [sb_([32, HALF * 256]) for _ in range(2)]

    # ---------------- PSUM ----------------
    gp0 = ps_([128, 512])     # layer-0 step gates, halves pp by t%2
    gp1 = ps_([128, 512])     # layer-1
    bulkp0 = ps_([128, 512])  # layer-0 input bulk chunks, halves pp by m%2
    bulkp1 = ps_([128, 512])
    trp = ps_([128, 512])     # inT transposes, halves pp
    outp = ps_([128, 512], BF16)  # output transposes: (t%2)*256 + k*128, rows 0:32

    import concourse.tile as tile_mod

    with tile_mod.TileContext(nc) as tc_:
        dma = nc.sync

        # ------------ constant + init loads ------------
        for k in range(2):
            dma.dma_start(m0t[k][:, :], m0t_d[128 * k:128 * (k + 1), :])
            dma.dma_start(whh0t[k][:, :], whh0t_d[128 * k:128 * (k + 1), :])
            dma.dma_start(whh1t[k][:, :], whh1t_d[128 * k:128 * (k + 1), :])
            dma.dma_start(wih1t[k][:, :], wih1t_d[128 * k:128 * (k + 1), :])
        dma.dma_start(beta0[:, :], beta0_d[:, :])
        dma.dma_start(beta1[:, :], beta1_d[:, :])
        dma.dma_start(ones[:, :], ones_d[:, :])
        dma.dma_start(id128f[:, :], id128f_d[:, :])
        dma.dma_start(id128r[:, :], id128r_d[:, :])
        dma.dma_start(id128b[:, :], id128b_d[:, :])
        # h rings: init goes to slot 15 (step 0 reads (0-1) % 16 = 15)
        for k in range(2):
            dma.dma_start(h0ring[:, 512 * k + 32 * 15:512 * k + 32 * 16],
                          h0t_init_d[128 * k:128 * (k + 1), :])
            dma.dma_start(h1ring[:, 512 * k + 32 * 15:512 * k + 32 * 16],
                          h1t_init_d[128 * k:128 * (k + 1), :])
        dma.dma_start(cT0[0][:, :], c0t_init_d[:, :])
        dma.dma_start(cT1[0][:, :], c1t_init_d[:, :])

        def emit_in_load(i, hb):
            """input DMA + transpose to input.T for one half-block."""
            src = inp[bass.ds(i + hb * (HALF * B), 1024), :].rearrange(
                "(r p) d -> p r d", p=128)
            dma.dma_start(inblk[hb % 2][:, :].rearrange("p (r d) -> p r d", r=8), src)
            for rr in range(8):
                for kd in range(2):
                    sl = trp[:, 128 * ((rr * 2 + kd) % 4):128 * ((rr * 2 + kd) % 4 + 1)]
                    nc.tensor.transpose(
                        sl,
                        inblk[hb % 2][:, 256 * rr + 128 * kd:256 * rr + 128 * (kd + 1)],
                        id128f[:, :],
                    )
                    dst = intp[kd][hb % 2][:, 128 * rr:128 * (rr + 1)]
                    if (rr + kd) % 2 == 0:
                        nc.scalar.copy(dst, sl)
                    else:
                        nc.vector.tensor_copy(dst, sl)

        def emit_bulk(layer, g8):
            """Bulk input-side gates for sub-block g8 (8 steps), transposed.

            layer 0: moving = intp (raw input, fused M0), layer 1: moving =
            h0ring slots of sub-block g8.
            """
            pp = g8 % 2
            if layer == 0:
                wt, bt, bp, gdst = m0t, beta0, bulkp0, gin0[pp]
                hb = (g8 // 4) % 2
                mv = [intp[k][hb][:, 256 * (g8 % 4):256 * (g8 % 4 + 1)]
                      for k in range(2)]
            else:
                wt, bt, bp, gdst = wih1t, beta1, bulkp1, gin1[pp]
                mv = [h0ring[:, 512 * k + 256 * pp:512 * k + 256 * (pp + 1)]
                      for k in range(2)]
            for m in range(8):
                out = bp[:, 256 * (m % 2):256 * (m % 2 + 1)]
                nc.tensor.matmul(out, bt[:, 128 * m:128 * (m + 1)],
                                 ones[:, 0:256], start=True, stop=False)
                for k in range(2):
                    nc.tensor.matmul(
                        out, wt[k][:, 128 * m:128 * (m + 1)], mv[k],
                        start=False, stop=(k == 1),
                    )
                if m % 2 == 1:
                    # evac chunk pair (m-1, m) -> gin cols (t, m', b);
                    # gpsimd can't read PSUM, so DVE (ACT is the hot engine)
                    dst = gdst[:, :].rearrange(
                        "p (t m b) -> p t m b", t=8, m=8)[:, :, m - 1:m + 1, :]
                    srcr = bp[:, :].rearrange("p (m t b) -> p t m b", m=2, t=8)
                    nc.vector.tensor_copy(dst, srcr)

        def layer_ctx(layer, t):
            p = t % 2
            if layer == 0:
                gp, ginb, whht, ring, cc, ss = (
                    gp0, gin0[(t // 8) % 2], whh0t, h0ring, cT0, s0)
            else:
                gp, ginb, whht, ring, cc, ss = (
                    gp1, gin1[(t // 8) % 2], whh1t, h1ring, cT1, s1)
            return p, gp[:, 256 * p:256 * (p + 1)], ginb, whht, ring, cc, ss

        def st_pe(layer, t):
            """Seed + recurrent matmuls for one layer-step."""
            p, g, ginb, whht, ring, cc, ss = layer_ctx(layer, t)
            prev = (t - 1) % NSLOT
            nc.tensor.matmul(
                g, id128r[:, :], ginb[:, 256 * (t % 8):256 * (t % 8 + 1)],
                start=True, stop=False, skip_group_check=True)
            for m in range(8):
                for k in range(2):
                    nc.tensor.matmul(
                        g[:, 32 * m:32 * (m + 1)],
                        whht[k][:, 128 * m:128 * (m + 1)],
                        ring[:, 512 * k + 32 * prev:512 * k + 32 * (prev + 1)],
                        start=False, stop=(k == 1), skip_group_check=True)

        def st_sig(layer, t):
            p, g, ginb, whht, ring, cc, ss = layer_ctx(layer, t)
            nc.scalar.activation(ss[p][:, :], g, AF.Sigmoid)

        def st_fc(layer, t, mybir_):
            """fc = f * c on Pool (all-SBUF operands)."""
            p, g, ginb, whht, ring, cc, ss = layer_ctx(layer, t)
            nc.gpsimd.tensor_mul(fc[layer][p][:, :], ss[p][:, 64:128],
                                 cc[t % 2][:, :])

        def st_igc(layer, t, mybir_):
            """ig then c_new on DVE."""
            p, g, ginb, whht, ring, cc, ss = layer_ctx(layer, t)
            s_ = ss[p]
            nc.vector.scalar_tensor_tensor(
                ig[layer][p][:, :], s_[:, 128:192], 0.5, s_[:, 0:64],
                mybir_.AluOpType.subtract, mybir_.AluOpType.mult)
            nc.vector.scalar_tensor_tensor(
                cc[(t + 1) % 2][:, :], ig[layer][p][:, :], 2.0,
                fc[layer][p][:, :],
                mybir_.AluOpType.mult, mybir_.AluOpType.add)

        def st_tanh(layer, t):
            p, g, ginb, whht, ring, cc, ss = layer_ctx(layer, t)
            nc.scalar.activation(tc[layer][p][:, :], cc[(t + 1) % 2][:, :],
                                 AF.Tanh)

        def st_hmul(layer, t, mybir_):
            """h.T = o * tanh(c) into the ring (bf16) on Pool."""
            p, g, ginb, whht, ring, cc, ss = layer_ctx(layer, t)
            slot = t % NSLOT
            dst = ring[:, :].rearrange(
                "p (k s b) -> p k s b", k=2, s=NSLOT)[:, :, slot, :]
            nc.gpsimd.tensor_mul(
                dst, ss[p][:, 192:256].rearrange("p (k b) -> p k b", k=2),
                tc[layer][p][:, :].rearrange("p (k b) -> p k b", k=2))

        def st_out_tr(t):
            """PE transposes of h1.T(t) into outp (deferred one step so PE
            never stalls on the just-computed hmul)."""
            p = t % 2
            slot = t % NSLOT
            for k in range(2):
                sl = outp[0:32, 256 * p + 128 * k:256 * p + 128 * (k + 1)]
                nc.tensor.transpose(
                    sl, h1ring[:, 512 * k + 32 * slot:512 * k + 32 * (slot + 1)],
                    id128b[:, :])

        def st_out_evac(t):
            """outp -> outblk [32, 256], on DVE."""
            p = t % 2
            hb_o = (t // HALF) % 2
            col = t % HALF
            dst_o = outblk[hb_o][:, 256 * col:256 * (col + 1)]
            nc.vector.tensor_copy(dst_o, outp[0:32, 256 * p:256 * (p + 1)])

        def emit_out_dma(i, off_steps, hb_o):
            """DMA one finished half-block of layer-1 output."""
            src = outblk[hb_o][:, :].rearrange("b (t d) -> b t d", t=HALF)
            dst = fwd[bass.ds(i + off_steps * B, HALF * B), :].rearrange(
                "(t b) d -> b t d", b=32)
            dma.dma_start(dst, src)

        def emit_pair(t0, t1, t1p, t1p_rel, i, mybir_):
            """Engine-interleaved emission of L0 step t0 and L1 step t1;
            t1p is the previous L1 step (mod BODY) whose output transpose/
            evac was deferred, t1p_rel its signed body-relative index for
            DMA addressing. Any of t0/t1/t1p may be None."""
            # --- PE ---
            if t1p is not None:
                st_out_tr(t1p)
            if t0 is not None:
                st_pe(0, t0)
            if t1 is not None:
                st_pe(1, t1)
            # --- ACT: sigmoids back to back, then tanhs (waits absorbed
            # by the other layer's work) ---
            if t0 is not None:
                st_sig(0, t0)
            if t1 is not None:
                st_sig(1, t1)
            # --- Pool ---
            if t0 is not None:
                st_fc(0, t0, mybir_)
            if t1 is not None:
                st_fc(1, t1, mybir_)
            # --- DVE phase 1 ---
            if t0 is not None:
                st_igc(0, t0, mybir_)
            if t1 is not None:
                st_igc(1, t1, mybir_)
            # --- ACT tanhs ---
            if t0 is not None:
                st_tanh(0, t0)
            if t1 is not None:
                st_tanh(1, t1)
            # --- DVE phase 2 ---
            if t1p is not None:
                st_out_evac(t1p)
            if t0 is not None:
                st_hmul(0, t0, mybir_)
            if t1 is not None:
                st_hmul(1, t1, mybir_)
            # --- DMA: half-block of L1 output complete at t1p ---
            if t1p_rel is not None and (t1p_rel % HALF) == HALF - 1:
                emit_out_dma(i, t1p_rel - (HALF - 1), ((t1p % BODY) // HALF) % 2)

        def emit_body_steady(i, mybir_):
            """Steady-state body: L1 steps t1 in [-8, 56) relative to i."""
            for sbk in range(8):
                hb = sbk // 4
                if sbk % 4 == 0:
                    emit_in_load(i, hb)
                emit_bulk(1, (sbk - 1) % 8)
                emit_bulk(0, sbk)
                for j in range(SB):
                    t0 = sbk * 8 + j
                    t1 = (t0 - 8) % BODY
                    t1p = (t0 - 9) % BODY
                    emit_pair(t0, t1, t1p, t0 - 9, i, mybir_)

        def emit_head(mybir_):
            """Body 0: no L1 work for its first 8 steps."""
            for sbk in range(8):
                hb = sbk // 4
                if sbk % 4 == 0:
                    emit_in_load(0, hb)
                if sbk >= 1:
                    emit_bulk(1, sbk - 1)
                emit_bulk(0, sbk)
                for j in range(SB):
                    t0 = sbk * 8 + j
                    t1 = t0 - 8 if t0 >= 8 else None
                    t1p = t0 - 9 if t0 >= 9 else None
                    emit_pair(t0, t1, t1p, t1p, 0, mybir_)

        def emit_epilogue(i_last, mybir_):
            """Flush L1's final 8 steps (sub-block 7 of the last body)."""
            emit_bulk(1, 7)
            for j in range(SB):
                t1 = 56 + j
                emit_pair(None, t1, t1 - 1, t1 - 1, i_last, mybir_)
            # final deferred output transpose/evac/DMA for step 63
            st_out_tr(63)
            st_out_evac(63)
            emit_out_dma(i_last, 32, 1)

        import concourse.mybir as mybir_mod

        for _rep in range(reps):
            if n_body == 1:
                emit_head(mybir_mod)
                emit_epilogue(0, mybir_mod)
            else:
                emit_head(mybir_mod)
                if n_body > 2:
                    with tc_.For_i(BODY * B, (n_body - 1) * BODY * B,
                                   BODY * B) as i:
                        emit_body_steady(i, mybir_mod)
                emit_body_steady((n_body - 1) * BODY * B, mybir_mod)
                emit_epilogue((n_body - 1) * BODY * B, mybir_mod)

    for cm in reversed(ctxs):
        cm.__exit__(None, None, None)

    nc.compile()
    return nc


def rne11(x):
    """Round fp32 to f32r: round-to-nearest-even keeping 11 mantissa bits."""
    xi = np.ascontiguousarray(x, np.float32).view(np.uint32).astype(np.uint64)
    shift = 12
    half = np.uint64(1 << (shift - 1))
    lsb = (xi >> np.uint64(shift)) & np.uint64(1)
    r = ((xi + half - np.uint64(1) + lsb) >> np.uint64(shift)) << np.uint64(shift)
    return (r & np.uint64(0xFFFFFFFF)).astype(np.uint32).view(np.float32).reshape(np.shape(x))


def prep_inputs(inputs, T):
    """Host-side input re-layout (weights only; input passed as-is)."""
    inp = np.ascontiguousarray(inputs["input"], dtype=np.float32)
    Wih = inputs["Wih"].astype(np.float64)
    Whh = inputs["Whh"].astype(np.float64)
    bih = inputs["bih"].astype(np.float64)
    bhh = inputs["bhh"].astype(np.float64)
    W_init = inputs["W_init"].astype(np.float64)
    b_init = inputs["b_init"].astype(np.float64)
    h0 = inputs["h0"].astype(np.float32)
    c0 = inputs["c0"].astype(np.float32)

    def g2(w):
        w = w.copy()
        w[512:768] *= 2.0
        return w

    Wih0g = g2(Wih[0])
    M0 = Wih0g @ W_init                      # [1024, 256]
    beta0 = Wih0g @ b_init + g2(bih[0] + bhh[0])
    beta1 = g2(bih[1] + bhh[1])

    def tT(w):   # [1024, 256] -> [256, 1024] f32r
        return rne11(np.ascontiguousarray(w.T).astype(np.float32))

    def cT(c):   # [32, 256] -> [128, 64] (col k*32+b = c[b, k*128+p])
        r = c.T.reshape(2, 128, 32).transpose(1, 0, 2).reshape(128, 64)
        return np.ascontiguousarray(r)

    import ml_dtypes
    bf16 = ml_dtypes.bfloat16

    def tB(w):   # [1024, 256] -> [256, 1024] bf16
        return np.ascontiguousarray(w.T).astype(np.float32).astype(bf16)

    im = {
        "input": inp.reshape(T * B, D),
        "m0t": tT(M0),
        "whh0t": tB(g2(Whh[0])),
        "whh1t": tB(g2(Whh[1])),
        "wih1t": tB(g2(Wih[1])),
        "beta0": rne11(beta0.astype(np.float32).reshape(1, 1024)),
        "beta1": rne11(beta1.astype(np.float32).reshape(1, 1024)),
        "ones": np.ones((1, 512), np.float32),
        "id128f": np.eye(128, dtype=np.float32),
        "id128r": np.eye(128, dtype=np.float32),
        "id128b": np.eye(128, dtype=np.float32).astype(bf16),
        "h0t_init": np.ascontiguousarray(h0[0].T).astype(bf16),
        "h1t_init": np.ascontiguousarray(h0[1].T).astype(bf16),
        "c0t_init": cT(c0[0]),
        "c1t_init": cT(c0[1]),
    }
    return im


def run_device(inputs, T, trace=False, repeats=0):
    import time
    from concourse import bass_utils

    nc = build(T)
    im = prep_inputs(inputs, T)
    res = bass_utils.run_bass_kernel_spmd(nc, [im], [0])
    times = []
    if trace or repeats:
        for _ in range(max(repeats, 3)):
            t0 = time.time()
            res = bass_utils.run_bass_kernel_spmd(nc, [im], [0])
            times.append(time.time() - t0)
        res.exec_time_ns = int(min(times) * 1e9)
    fwd = res.results[0]["fwd"].reshape(T, B, H)
    return fwd, res


def kernel(**inputs):
    T = inputs["input"].shape[0]
    fwd, _ = run_device(inputs, T)
    out = np.empty((T, B, 2 * H), dtype=np.float32)
    out[:, :, :H] = fwd
    out[:, :, H:] = fwd[-1][None]
    return out


if __name__ == "__main__":
    # CoreSim smoke test with small T
    from concourse.bass_interp import CoreSim

    T = int(os.environ.get("SIM_T", "64"))
    rng = np.random.default_rng(0)
    k = 1.0 / np.sqrt(H)
    inputs = {
        "input": rng.standard_normal((T, B, D), dtype=np.float32),
        "W_init": rng.uniform(-k, k, (H, D)).astype(np.float32),
        "b_init": rng.uniform(-k, k, (H,)).astype(np.float32),
        "Wih": rng.uniform(-k, k, (2, 4 * H, H)).astype(np.float32),
        "Whh": rng.uniform(-k, k, (2, 4 * H, H)).astype(np.float32),
        "bih": rng.uniform(-k, k, (2, 4 * H)).astype(np.float32),
        "bhh": rng.uniform(-k, k, (2, 4 * H)).astype(np.float32),
        "h0": rng.uniform(-k, k, (2, B, H)).astype(np.float32),
        "c0": rng.uniform(-k, k, (2, B, H)).astype(np.float32),
    }

    def np_ref(inp):
        x_all = inp["input"]
        h = inp["h0"].copy()
        c = inp["c0"].copy()
        outs = []
        for t in range(T):
            x = x_all[t] @ inp["W_init"].T + inp["b_init"]
            for l in range(2):
                gates = (x @ inp["Wih"][l].T + inp["bih"][l]
                         + h[l] @ inp["Whh"][l].T + inp["bhh"][l])
                i_, f_, g_, o_ = np.split(gates, 4, axis=-1)
                i_ = 1 / (1 + np.exp(-i_)); f_ = 1 / (1 + np.exp(-f_))
                o_ = 1 / (1 + np.exp(-o_)); g_ = np.tanh(g_)
                c[l] = f_ * c[l] + i_ * g_
                h[l] = o_ * np.tanh(c[l])
                x = h[l]
            outs.append(h[1].copy())
        return np.stack(outs)

    expected = np_ref(inputs)
    nc = build(T)
    sim = CoreSim(nc)
    im = prep_inputs(inputs, T)
    for name, arr in im.items():
        sim.tensor(name)[:] = arr
    sim.simulate()
    got = sim.tensor("fwd").reshape(T, B, H)
    err = np.abs(got - expected).max() / (np.abs(expected).max() + 1e-9)
    print("SIM time:", sim.time, "ns  per-step:", sim.time / T)
    print("SIM max-rel err:", err)
    print("sample got", got[0, 0, :4], "exp", expected[0, 0, :4])
